# revision 39
# baseline (speedup 1.0000x reference)
"""GQA causal self-attention (B=2, T=2048, C=2048, 16 Q heads / 4 KV heads,
head_dim=128) on 8 TRN2 NeuronCores.

Sharding: core = (batch b, kv-group g) for b in {0,1}, g in {0..3}.
Each core computes its batch's 4 Q heads that share KV head g, plus the
partial out-projection over those heads' rows of W_out. Host sums the 4
partials per batch and adds b_out.

v2 layout/engine choices:
  - q/k projection in fp8e4 DoubleRow (weights pre-scaled x64 on host,
    un-scaled in the ACT PSUM-drain which also adds the bias); v stays bf16.
  - v [d,t] -> [t,d] transposes via DMA-transpose (no PE).
  - attention jt-outer: per key-tile, all 4 heads share the k/v stationary
    operand; av matmuls software-pipelined 2 tiles behind the scores so PE
    never waits for ACT's exp.
  - softmax denominator off PE: DVE accumulates P tiles (bf16), gpsimd
    partition_all_reduce gives the broadcast row-sum, DVE fast-reciprocal
    and multiply produce normalized y^T.
  - out-projection for slice s interleaved into slice s+1's first two key
    tiles; PSUM: scores(2) + psy(4) + outproj/swap(2) banks.
"""

import sys

if "/opt/trn_rl_repo" not in sys.path:
    sys.path.insert(0, "/opt/trn_rl_repo")

import numpy as np
import ml_dtypes

BF16 = ml_dtypes.bfloat16

B = 2
T = 2048
C = 2048
NH = 16
NKV = 4
D = 128
GQ = NH // NKV  # 4 q heads per kv head
N_CORES = 8
CC = C // 128  # 16 contraction chunks
TS = T // 512  # 4 t-slices
TT = T // 128  # 16 t-tiles
NQK = GQ + 1  # fp8 feature chunks per core: 4 q heads + k
WSCALE = 64.0  # fp8 weight pre-scale

_CACHED = {}


def _build_bass(reps=1):
    import concourse.bass as bass
    import concourse.bacc as bacc
    import concourse.tile as tile
    import concourse.mybir as mybir

    bf = mybir.dt.bfloat16
    f32 = mybir.dt.float32
    f8 = mybir.dt.float8e4

    nc = bacc.Bacc(None, target_bir_lowering=False)

    xt8_d = nc.dram_tensor("xt8", [128, TS, CC, 512], f8, kind="ExternalInput")
    xtb_d = nc.dram_tensor("xtb", [128, TS, CC, 512], bf, kind="ExternalInput")
    wqk8_d = nc.dram_tensor("wqk8", [128, CC, NQK * 128], f8, kind="ExternalInput")
    wv_d = nc.dram_tensor("wv", [128, CC, 128], bf, kind="ExternalInput")
    bqkv_d = nc.dram_tensor("bqkv", [128, NQK + 1], f32, kind="ExternalInput")
    cos_d = nc.dram_tensor("cosT", [128, T], bf, kind="ExternalInput")
    sin_d = nc.dram_tensor("sinT", [128, T], bf, kind="ExternalInput")
    swap_d = nc.dram_tensor("swp", [128, 128], bf, kind="ExternalInput")
    iden_d = nc.dram_tensor("idn", [128, 128], bf, kind="ExternalInput")
    wout_d = nc.dram_tensor("wout", [128, GQ, C], bf, kind="ExternalInput")
    out_d = nc.dram_tensor("out", [T, C], f32, kind="ExternalOutput")

    with tile.TileContext(nc) as tc:
        with (
            tc.tile_pool(name="persist", bufs=1) as pers,
            tc.tile_pool(name="xt", bufs=2) as xtp,
            tc.tile_pool(name="stage", bufs=4) as stg,
            tc.tile_pool(name="ptile", bufs=8) as ptp,
            tc.tile_pool(name="pacc", bufs=2) as pap,
            tc.tile_pool(name="small", bufs=2) as smp,
            tc.tile_pool(name="osb", bufs=3) as osp,
            tc.tile_pool(name="ps_a", bufs=3, space="PSUM") as ppa,
            tc.tile_pool(name="ps_y", bufs=1, space="PSUM") as ppy,
            tc.tile_pool(name="ps_o", bufs=3, space="PSUM") as ppo,
        ):
            import contextlib
            loop_cm = tc.For_i(0, reps, 1) if reps > 1 else contextlib.nullcontext()
            with loop_cm:
                _body(nc, tc, mybir, bf, f32, f8,
                      pers, xtp, stg, ptp, pap, smp, osp, ppa, ppy, ppo,
                      xt8_d, xtb_d, wqk8_d, wv_d, bqkv_d, cos_d, sin_d, swap_d,
                      iden_d, wout_d, out_d)
    nc.compile()
    return nc


def _body(nc, tc, mybir, bf, f32, f8,
          pers, xtp, stg, ptp, pap, smp, osp, ppa, ppy, ppo,
          xt8_d, xtb_d, wqk8_d, wv_d, bqkv_d, cos_d, sin_d, swap_d,
          iden_d, wout_d, out_d):
    Exp = mybir.ActivationFunctionType.Exp
    Ident = mybir.ActivationFunctionType.Identity
    DR = mybir.MatmulPerfMode.DoubleRow
    mul = mybir.AluOpType.mult
    add = mybir.AluOpType.add

    # ---- persistent loads, in need-order ----
    wqk_sb = pers.tile([128, CC, NQK * 128], f8)
    xt8_0 = xtp.tile([128, CC, 512], f8, tag="xt8")
    # spread startup loads over the three DMA-capable queues: the q/k
    # critical path on sync, x-bf16/wv on scalar, tables/wout on gpsimd
    bq_sb = pers.tile([128, NQK + 1], f32)
    nc.gpsimd.dma_start(bq_sb[:], bqkv_d[:])
    xtb_0 = xtp.tile([128, CC, 512], bf, tag="xtb")
    nc.scalar.dma_start(xtb_0[:], xtb_d[:, 0])
    wv_sb = pers.tile([128, CC, 128], bf)
    nc.scalar.dma_start(wv_sb[:], wv_d[:])
    # small first chunks so the first matmuls can start early, then big ones
    for q in (slice(0, 2), slice(2, 5), slice(5, 10), slice(10, 16)):
        nc.sync.dma_start(wqk_sb[:, q, :], wqk8_d[:, q, :])
        nc.sync.dma_start(xt8_0[:, q, :], xt8_d[:, 0, q, :])
    cos_sb = pers.tile([128, T], bf)
    nc.gpsimd.dma_start(cos_sb[:], cos_d[:])
    sin_sb = pers.tile([128, T], bf)
    nc.gpsimd.dma_start(sin_sb[:], sin_d[:])
    swap_sb = pers.tile([128, 128], bf)
    nc.gpsimd.dma_start(swap_sb[:], swap_d[:])
    iden_sb = pers.tile([128, 128], bf)
    nc.gpsimd.dma_start(iden_sb[:], iden_d[:])
    wout_sb = pers.tile([128, GQ, C], bf)
    nc.gpsimd.dma_start(wout_sb[:], wout_d[:])
    ones_sb = pers.tile([128, 128], bf)
    nc.vector.memset(ones_sb[:], 1.0)

    # persistent activations
    qk_sb = pers.tile([128, NQK, T], bf)  # rotated q0..q3, k
    v_sb = pers.tile([128, TT, 128], bf)  # v in [t-part, d] tiles
    y_sb = pers.tile([128, GQ, T], bf)  # y^T per head

    # ---- emission streams: QKV slices, attention slices, out-projection ----
    # Work is emitted as lists of thunks; independent streams are merged by
    # proportional interleave so the PE-heavy QKV/out-proj work fills the
    # ACT-bound attention stretches (slice-level software pipeline).
    rope_q = []  # deferred swap-matmul chains: (raw, f, tsl)

    def emit_rope(raw, f, tsl):
        psw = ppo.tile([128, 512], f32, tag="ops")
        nc.tensor.matmul(psw[:], swap_sb[:], raw[:], start=True, stop=True)
        tmp = stg.tile([128, 512], bf, tag="ropetmp")
        nc.vector.tensor_tensor(tmp[:], psw[:], sin_sb[:, tsl], mul)
        nc.vector.tensor_tensor(qk_sb[:, f, tsl], raw[:], cos_sb[:, tsl], mul)
        nc.vector.tensor_tensor(qk_sb[:, f, tsl], qk_sb[:, f, tsl], tmp[:], add)

    xt_cur = {0: (xt8_0, xtb_0)}

    def qkv_stream(ts):
        tsl = slice(ts * 512, (ts + 1) * 512)
        st = {}
        thunks = []

        def fetch():
            xt8, xtb = xt_cur.pop(ts)
            st["xt"] = (xt8, xtb)
            if ts + 1 < TS:
                nxt8 = xtp.tile([128, CC, 512], f8, tag="xt8", name="xt8")
                nc.sync.dma_start(nxt8[:], xt8_d[:, ts + 1])
                nxtb = xtp.tile([128, CC, 512], bf, tag="xtb", name="xtb")
                nc.scalar.dma_start(nxtb[:], xtb_d[:, ts + 1])
                xt_cur[ts + 1] = (nxt8, nxtb)

        thunks.append(fetch)

        def qk_chunk(f):
            xt8 = st["xt"][0]
            ps = ppa.tile([128, 512], f32, tag="aps", name="qkps")
            for c2 in range(CC // 2):
                nc.tensor.matmul(
                    ps[:],
                    wqk_sb[:, 2 * c2 : 2 * c2 + 2, f * 128 : (f + 1) * 128],
                    xt8[:, 2 * c2 : 2 * c2 + 2, :],
                    start=(c2 == 0),
                    stop=(c2 == CC // 2 - 1),
                    perf_mode=DR,
                )
            raw = stg.tile([128, 512], bf, tag="raw", name="raw")
            nc.scalar.activation(
                raw[:], ps[:], Ident,
                bias=bq_sb[:, f : f + 1], scale=1.0 / WSCALE,
            )
            if rope_q:
                emit_rope(*rope_q.pop(0))
            rope_q.append((raw, f, tsl))

        for f in range(NQK):
            thunks.append(lambda f=f: qk_chunk(f))

        def v_half(q):
            xtb = st["xt"][1]
            if q == 0:
                st["ps"] = ppa.tile([128, 512], f32, tag="aps", name="vps")
            ps = st["ps"]
            for cc in range(8 * q, 8 * q + 8):
                nc.tensor.matmul(
                    ps[:],
                    wv_sb[:, cc, :],
                    xtb[:, cc, :],
                    start=(cc == 0),
                    stop=(cc == CC - 1),
                )
            if q == 1:
                vraw = stg.tile([128, 512], bf, tag="raw", name="vraw")
                nc.scalar.activation(
                    vraw[:], ps[:], Ident, bias=bq_sb[:, NQK : NQK + 1], scale=1.0
                )
                st["vraw"] = vraw
                if rope_q:
                    emit_rope(*rope_q.pop(0))

        for q in range(2):
            thunks.append(lambda q=q: v_half(q))

        def vtrans():
            vraw = st["vraw"]
            pvt = ppo.tile([128, 512], bf, tag="ops", name="pvt")
            for k in range(4):
                nc.tensor.transpose(
                    pvt[:, k * 128 : (k + 1) * 128],
                    vraw[:, k * 128 : (k + 1) * 128],
                    iden_sb[:],
                )
            nc.vector.tensor_copy(v_sb[:, ts * 4 : ts * 4 + 4, :], pvt[:])
            if ts == TS - 1:
                while rope_q:
                    emit_rope(*rope_q.pop(0))

        thunks.append(vtrans)
        return thunks

    PIPE = 4  # av/den lag this many score tiles behind

    def att_stream(s):
        isl = slice(s * 512, (s + 1) * 512)
        njt = 4 * (s + 1)
        st = {"pend": []}
        thunks = []

        def start_h(h):
            st["psy"] = ppy.tile([128, 512], f32, tag=f"yps{h % 2}", name="psy")
            st[f"pacc{h}"] = pap.tile(
                [128, 512], bf, tag=f"pacc{h}", name=f"pacc{h}"
            )

        def drain_one():
            P, h, jt, off = st["pend"].pop(0)
            nc.tensor.matmul(
                st["psy"][:, off:512],
                v_sb[:, jt, :],
                P[:, off:512],
                start=(jt == 0),
                stop=(jt == njt - 1),
            )
            pacc = st[f"pacc{h}"]
            if jt == 0:
                nc.vector.tensor_copy(pacc[:], P[:])
            else:
                nc.vector.tensor_tensor(
                    pacc[:, off:512], pacc[:, off:512], P[:, off:512], add
                )

        def att_iter(h, jt):
            if jt == 0:
                start_h(h)
            off = max(0, 128 * jt - 512 * s)
            pss = ppa.tile([128, 512], f32, tag="aps", name="pss")
            nc.tensor.matmul(
                pss[:, off:512],
                qk_sb[:, GQ, jt * 128 : (jt + 1) * 128],
                qk_sb[:, h, s * 512 + off : (s + 1) * 512],
                start=True,
                stop=True,
            )
            P = ptp.tile([128, 512], bf, tag="P", name="P")
            nc.scalar.activation(
                P[:, off:512], pss[:, off:512], Exp, scale=1.0 / 128.0
            )
            if jt >= 4 * s:
                nc.gpsimd.affine_select(
                    out=P[:, off : off + 128],
                    in_=P[:, off : off + 128],
                    pattern=[[1, 128]],
                    compare_op=mybir.AluOpType.is_ge,
                    fill=0.0,
                    base=0,
                    channel_multiplier=-1,
                )
            while len(st["pend"]) >= PIPE:
                drain_one()
            st["pend"].append((P, h, jt, off))

        def normalize(h):
            while st["pend"]:
                drain_one()
            psd = ppo.tile([128, 512], f32, tag="ops", name="psd")
            nc.tensor.matmul(
                psd[:], ones_sb[:], st[f"pacc{h}"][:], start=True, stop=True
            )
            rinv = smp.tile([128, 512], f32, tag="rinv", name="rinv")
            nc.vector.reciprocal_approx_fast(rinv[:], psd[:])
            nc.vector.tensor_tensor(y_sb[:, h, isl], st["psy"][:], rinv[:], mul)

        for h in range(GQ):
            for jt in range(njt):
                thunks.append(lambda h=h, jt=jt: att_iter(h, jt))
            thunks.append(lambda h=h: normalize(h))
        return thunks

    def op_stream(s):
        st = {}
        thunks = []

        def group(tt, es):
            if es == 0:
                st["osb"] = osp.tile([128, C], f32, tag="osb", name="osb")
            pso = ppo.tile([128, 512], f32, tag="ops", name="pso")
            for h in range(GQ):
                nc.tensor.matmul(
                    pso[:],
                    y_sb[:, h, tt * 128 : (tt + 1) * 128],
                    wout_sb[:, h, es * 512 : (es + 1) * 512],
                    start=(h == 0),
                    stop=(h == GQ - 1),
                )
            o_sb = st["osb"]
            osl = slice(es * 512, (es + 1) * 512)
            if es % 2 == 0:
                nc.vector.tensor_copy(o_sb[:, osl], pso[:])
            else:
                nc.scalar.copy(o_sb[:, osl], pso[:])
            nc.sync.dma_start(out_d[tt * 128 : (tt + 1) * 128, osl], o_sb[:, osl])

        for tt in range(4 * s, 4 * s + 4):
            for es in range(4):
                thunks.append(lambda tt=tt, es=es: group(tt, es))
        return thunks

    def run_interleaved(*streams):
        # proportional (Bresenham) merge; streams are lists of thunks
        streams = [list(s) for s in streams if s]
        total = max(len(s) for s in streams)
        err = [0.0] * len(streams)
        idx = [0] * len(streams)
        for _ in range(total):
            for k, s in enumerate(streams):
                err[k] += len(s) / total
                while err[k] >= 1.0 and idx[k] < len(s):
                    s[idx[k]]()
                    idx[k] += 1
                    err[k] -= 1.0
        for k, s in enumerate(streams):
            while idx[k] < len(s):
                s[idx[k]]()
                idx[k] += 1

    # slice-level software pipeline: attention(s) overlaps QKV(s+1);
    # out-projections ride in the late (ACT-bound) attention slices
    run_interleaved(qkv_stream(0))
    run_interleaved(att_stream(0), qkv_stream(1))
    run_interleaved(att_stream(1), qkv_stream(2))
    run_interleaved(att_stream(2), qkv_stream(3), op_stream(0))
    run_interleaved(att_stream(3), op_stream(1), op_stream(2))
    run_interleaved(op_stream(3))


def _host_prep(x, rope_cache, W_qkv, b_qkv, W_out):
    """Build the 8 per-core input dicts."""
    import concourse.mybir as mybir

    F8 = mybir.dt.np(mybir.dt.float8e4)
    q_dim = NH * D  # 2048
    kv_dim = NKV * D  # 512

    # rope tables in [d, t] layout
    sin = rope_cache[:, 0::2].astype(np.float32)  # [T, 64]
    cos = rope_cache[:, 1::2].astype(np.float32)
    cos2T = np.empty((128, T), np.float32)
    sinsT = np.empty((128, T), np.float32)
    cos2T[0::2] = cos.T
    cos2T[1::2] = cos.T
    sinsT[0::2] = -sin.T
    sinsT[1::2] = sin.T
    cos2T = cos2T.astype(BF16)
    sinsT = sinsT.astype(BF16)

    swap = np.zeros((128, 128), BF16)
    idx = np.arange(128)
    swap[idx, idx ^ 1] = 1
    iden = np.eye(128, dtype=BF16)

    def tile_cols(w, ncols):
        # [C, ncols*128] -> [128, CC, ncols*128] with contraction c = cc*128+p
        return np.ascontiguousarray(
            w.reshape(CC, 128, ncols * 128).transpose(1, 0, 2)
        )

    in_maps = []
    for b in range(B):
        xT = np.ascontiguousarray(x[b].T)  # [C, T] f32
        xT = xT.reshape(CC, 128, T).transpose(1, 0, 2)  # [128, CC, T]
        # -> [128, TS, CC, 512]: per-slice DMA reads contiguous lines
        xT = xT.reshape(128, CC, TS, 512).transpose(0, 2, 1, 3)
        xt8 = np.ascontiguousarray(np.clip(xT, -240, 240).astype(F8))
        xtb = np.ascontiguousarray(xT.astype(BF16))
        for g in range(NKV):
            qk_cols = np.concatenate(
                [
                    np.arange(4 * g * D, (4 * g + 4) * D),  # 4 q heads
                    np.arange(q_dim + g * D, q_dim + (g + 1) * D),  # k head
                ]
            )
            v_cols = np.arange(q_dim + kv_dim + g * D, q_dim + kv_dim + (g + 1) * D)
            wqk = np.clip(W_qkv[:, qk_cols] * WSCALE, -240, 240).astype(F8)
            wv = W_qkv[:, v_cols].astype(BF16)
            bq = np.ascontiguousarray(
                b_qkv[np.concatenate([qk_cols, v_cols])]
                .astype(np.float32).reshape(NQK + 1, 128).T
            )  # [128, NQK+1]
            wo = W_out[4 * g * D : (4 * g + 4) * D, :].astype(BF16)  # [512, C]
            wo = np.ascontiguousarray(
                wo.reshape(GQ, 128, C).transpose(1, 0, 2)
            )  # [128, GQ, C]
            in_maps.append(
                {
                    "xt8": xt8,
                    "xtb": xtb,
                    "wqk8": tile_cols(wqk, NQK),
                    "wv": tile_cols(wv, 1),
                    "bqkv": bq,
                    "cosT": cos2T,
                    "sinT": sinsT,
                    "swp": swap,
                    "idn": iden,
                    "wout": wo,
                }
            )
    return in_maps


def kernel(x, rope_cache, W_qkv, b_qkv, W_out, b_out, _trace=False):
    from concourse.bass_utils import run_bass_kernel_spmd

    if "nc" not in _CACHED:
        _CACHED["nc"] = _build_bass()
    nc = _CACHED["nc"]

    in_maps = _host_prep(
        np.asarray(x), np.asarray(rope_cache), np.asarray(W_qkv),
        np.asarray(b_qkv), np.asarray(W_out),
    )
    res = run_bass_kernel_spmd(nc, in_maps, core_ids=list(range(N_CORES)), trace=_trace)
    _CACHED["last_result"] = res

    out = np.zeros((B, T, C), np.float32)
    for b in range(B):
        acc = res.results[b * NKV]["out"].astype(np.float32)
        for g in range(1, NKV):
            acc = acc + res.results[b * NKV + g]["out"]
        out[b] = acc + np.asarray(b_out)[None, :]
    return out


# revision 41
# speedup vs baseline: 1.0923x; 1.0923x over previous
"""GQA causal self-attention (B=2, T=2048, C=2048, 16 Q heads / 4 KV heads,
head_dim=128) on 8 TRN2 NeuronCores.

Sharding: core = (batch b, kv-group g) for b in {0,1}, g in {0..3}.
Each core computes its batch's 4 Q heads that share KV head g, plus the
partial out-projection over those heads' rows of W_out. Host sums the 4
partials per batch and adds b_out.

v2 layout/engine choices:
  - q/k projection in fp8e4 DoubleRow (weights pre-scaled x64 on host,
    un-scaled in the ACT PSUM-drain which also adds the bias); v stays bf16.
  - v [d,t] -> [t,d] transposes via DMA-transpose (no PE).
  - attention jt-outer: per key-tile, all 4 heads share the k/v stationary
    operand; av matmuls software-pipelined 2 tiles behind the scores so PE
    never waits for ACT's exp.
  - softmax denominator off PE: DVE accumulates P tiles (bf16), gpsimd
    partition_all_reduce gives the broadcast row-sum, DVE fast-reciprocal
    and multiply produce normalized y^T.
  - out-projection for slice s interleaved into slice s+1's first two key
    tiles; PSUM: scores(2) + psy(4) + outproj/swap(2) banks.
"""

import sys

if "/opt/trn_rl_repo" not in sys.path:
    sys.path.insert(0, "/opt/trn_rl_repo")

import numpy as np
import ml_dtypes

BF16 = ml_dtypes.bfloat16

B = 2
T = 2048
C = 2048
NH = 16
NKV = 4
D = 128
GQ = NH // NKV  # 4 q heads per kv head
N_CORES = 8
CC = C // 128  # 16 contraction chunks
TS = T // 512  # 4 t-slices
TT = T // 128  # 16 t-tiles
NQK = GQ + 1  # fp8 feature chunks per core: 4 q heads + k
WSCALE = 64.0  # fp8 weight pre-scale

_CACHED = {}


def _build_bass(reps=1):
    import concourse.bass as bass
    import concourse.bacc as bacc
    import concourse.tile as tile
    import concourse.mybir as mybir

    bf = mybir.dt.bfloat16
    f32 = mybir.dt.float32
    f8 = mybir.dt.float8e4

    nc = bacc.Bacc(None, target_bir_lowering=False)

    xt8_d = nc.dram_tensor("xt8", [128, TS, CC, 512], f8, kind="ExternalInput")
    xtb_d = nc.dram_tensor("xtb", [128, TS, CC, 512], bf, kind="ExternalInput")
    wqk8_d = nc.dram_tensor("wqk8", [128, CC, NQK * 128], f8, kind="ExternalInput")
    wv_d = nc.dram_tensor("wv", [128, CC, 128], bf, kind="ExternalInput")
    bqkv_d = nc.dram_tensor("bqkv", [128, NQK + 1], f32, kind="ExternalInput")
    cos_d = nc.dram_tensor("cosT", [128, T], bf, kind="ExternalInput")
    sin_d = nc.dram_tensor("sinT", [128, T], bf, kind="ExternalInput")
    swap_d = nc.dram_tensor("swp", [128, 128], bf, kind="ExternalInput")
    iden_d = nc.dram_tensor("idn", [128, 128], bf, kind="ExternalInput")
    wout_d = nc.dram_tensor("wout", [128, GQ, C], bf, kind="ExternalInput")
    out_d = nc.dram_tensor("out", [T, C], f32, kind="ExternalOutput")

    with tile.TileContext(nc) as tc:
        with (
            tc.tile_pool(name="persist", bufs=1) as pers,
            tc.tile_pool(name="xt", bufs=2) as xtp,
            tc.tile_pool(name="stage", bufs=4) as stg,
            tc.tile_pool(name="ptile", bufs=8) as ptp,
            tc.tile_pool(name="pacc", bufs=2) as pap,
            tc.tile_pool(name="small", bufs=2) as smp,
            tc.tile_pool(name="osb", bufs=3) as osp,
            tc.tile_pool(name="ps_a", bufs=3, space="PSUM") as ppa,
            tc.tile_pool(name="ps_y", bufs=1, space="PSUM") as ppy,
            tc.tile_pool(name="ps_o", bufs=3, space="PSUM") as ppo,
        ):
            import contextlib
            loop_cm = tc.For_i(0, reps, 1) if reps > 1 else contextlib.nullcontext()
            with loop_cm:
                _body(nc, tc, mybir, bf, f32, f8,
                      pers, xtp, stg, ptp, pap, smp, osp, ppa, ppy, ppo,
                      xt8_d, xtb_d, wqk8_d, wv_d, bqkv_d, cos_d, sin_d, swap_d,
                      iden_d, wout_d, out_d)
    nc.compile()
    return nc


def _body(nc, tc, mybir, bf, f32, f8,
          pers, xtp, stg, ptp, pap, smp, osp, ppa, ppy, ppo,
          xt8_d, xtb_d, wqk8_d, wv_d, bqkv_d, cos_d, sin_d, swap_d,
          iden_d, wout_d, out_d):
    Exp = mybir.ActivationFunctionType.Exp
    Ident = mybir.ActivationFunctionType.Identity
    DR = mybir.MatmulPerfMode.DoubleRow
    mul = mybir.AluOpType.mult
    add = mybir.AluOpType.add

    # ---- persistent loads, in need-order ----
    wqk_sb = pers.tile([128, CC, NQK * 128], f8)
    xt8_0 = xtp.tile([128, CC, 512], f8, tag="xt8")
    # small first chunks so the first matmuls can start early, then big ones
    for q in (slice(0, 2), slice(2, 5), slice(5, 10), slice(10, 16)):
        nc.sync.dma_start(wqk_sb[:, q, :], wqk8_d[:, q, :])
        nc.sync.dma_start(xt8_0[:, q, :], xt8_d[:, 0, q, :])
    bq_sb = pers.tile([128, NQK + 1], f32)
    nc.sync.dma_start(bq_sb[:], bqkv_d[:])
    swap_sb = pers.tile([128, 128], bf)
    nc.sync.dma_start(swap_sb[:], swap_d[:])
    iden_sb = pers.tile([128, 128], bf)
    nc.sync.dma_start(iden_sb[:], iden_d[:])
    cos_sb = pers.tile([128, T], bf)
    nc.sync.dma_start(cos_sb[:], cos_d[:])
    sin_sb = pers.tile([128, T], bf)
    nc.sync.dma_start(sin_sb[:], sin_d[:])
    wv_sb = pers.tile([128, CC, 128], bf)
    nc.sync.dma_start(wv_sb[:], wv_d[:])
    xtb_0 = xtp.tile([128, CC, 512], bf, tag="xtb")
    nc.sync.dma_start(xtb_0[:], xtb_d[:, 0])
    wout_sb = pers.tile([128, GQ, C], bf)
    nc.sync.dma_start(wout_sb[:], wout_d[:])
    ones_sb = pers.tile([128, 128], bf)
    nc.vector.memset(ones_sb[:], 1.0)

    # persistent activations
    qk_sb = pers.tile([128, NQK, T], bf)  # rotated q0..q3, k
    v_sb = pers.tile([128, TT, 128], bf)  # v in [t-part, d] tiles
    y_sb = pers.tile([128, GQ, T], bf)  # y^T per head

    # ---- emission streams: QKV slices, attention slices, out-projection ----
    # Work is emitted as lists of thunks; independent streams are merged by
    # proportional interleave so the PE-heavy QKV/out-proj work fills the
    # ACT-bound attention stretches (slice-level software pipeline).
    rope_q = []  # deferred swap-matmul chains: (raw, f, tsl)

    def emit_rope(raw, f, tsl):
        psw = ppo.tile([128, 512], f32, tag="ops")
        nc.tensor.matmul(psw[:], swap_sb[:], raw[:], start=True, stop=True)
        tmp = stg.tile([128, 512], bf, tag="ropetmp")
        nc.vector.tensor_tensor(tmp[:], psw[:], sin_sb[:, tsl], mul)
        nc.vector.tensor_tensor(qk_sb[:, f, tsl], raw[:], cos_sb[:, tsl], mul)
        nc.vector.tensor_tensor(qk_sb[:, f, tsl], qk_sb[:, f, tsl], tmp[:], add)

    xt_cur = {0: (xt8_0, xtb_0)}

    def qkv_stream(ts):
        tsl = slice(ts * 512, (ts + 1) * 512)
        st = {}
        thunks = []

        def fetch():
            xt8, xtb = xt_cur.pop(ts)
            st["xt"] = (xt8, xtb)
            if ts + 1 < TS:
                nxt8 = xtp.tile([128, CC, 512], f8, tag="xt8", name="xt8")
                nc.sync.dma_start(nxt8[:], xt8_d[:, ts + 1])
                nxtb = xtp.tile([128, CC, 512], bf, tag="xtb", name="xtb")
                nc.sync.dma_start(nxtb[:], xtb_d[:, ts + 1])
                xt_cur[ts + 1] = (nxt8, nxtb)

        thunks.append(fetch)

        def qk_chunk(f):
            xt8 = st["xt"][0]
            ps = ppa.tile([128, 512], f32, tag="aps", name="qkps")
            for c2 in range(CC // 2):
                nc.tensor.matmul(
                    ps[:],
                    wqk_sb[:, 2 * c2 : 2 * c2 + 2, f * 128 : (f + 1) * 128],
                    xt8[:, 2 * c2 : 2 * c2 + 2, :],
                    start=(c2 == 0),
                    stop=(c2 == CC // 2 - 1),
                    perf_mode=DR,
                )
            raw = stg.tile([128, 512], bf, tag="raw", name="raw")
            nc.scalar.activation(
                raw[:], ps[:], Ident,
                bias=bq_sb[:, f : f + 1], scale=1.0 / WSCALE,
            )
            if rope_q:
                emit_rope(*rope_q.pop(0))
            rope_q.append((raw, f, tsl))

        for f in range(NQK):
            thunks.append(lambda f=f: qk_chunk(f))

        def v_half(q):
            xtb = st["xt"][1]
            if q == 0:
                st["ps"] = ppa.tile([128, 512], f32, tag="aps", name="vps")
            ps = st["ps"]
            for cc in range(8 * q, 8 * q + 8):
                nc.tensor.matmul(
                    ps[:],
                    wv_sb[:, cc, :],
                    xtb[:, cc, :],
                    start=(cc == 0),
                    stop=(cc == CC - 1),
                )
            if q == 1:
                vraw = stg.tile([128, 512], bf, tag="raw", name="vraw")
                nc.scalar.activation(
                    vraw[:], ps[:], Ident, bias=bq_sb[:, NQK : NQK + 1], scale=1.0
                )
                st["vraw"] = vraw
                if rope_q:
                    emit_rope(*rope_q.pop(0))

        for q in range(2):
            thunks.append(lambda q=q: v_half(q))

        def vtrans():
            vraw = st["vraw"]
            pvt = ppo.tile([128, 512], bf, tag="ops", name="pvt")
            for k in range(4):
                nc.tensor.transpose(
                    pvt[:, k * 128 : (k + 1) * 128],
                    vraw[:, k * 128 : (k + 1) * 128],
                    iden_sb[:],
                )
            nc.vector.tensor_copy(v_sb[:, ts * 4 : ts * 4 + 4, :], pvt[:])
            if ts == TS - 1:
                while rope_q:
                    emit_rope(*rope_q.pop(0))

        thunks.append(vtrans)
        return thunks

    PIPE = 4  # av/den lag this many score tiles behind

    def att_stream(s):
        isl = slice(s * 512, (s + 1) * 512)
        njt = 4 * (s + 1)
        st = {"pend": []}
        thunks = []

        def start_h(h):
            st["psy"] = ppy.tile([128, 512], f32, tag=f"yps{h % 2}", name="psy")
            st[f"pacc{h}"] = pap.tile(
                [128, 512], bf, tag=f"pacc{h}", name=f"pacc{h}"
            )

        def drain_one():
            P, h, jt, off = st["pend"].pop(0)
            nc.tensor.matmul(
                st["psy"][:, off:512],
                v_sb[:, jt, :],
                P[:, off:512],
                start=(jt == 0),
                stop=(jt == njt - 1),
            )
            pacc = st[f"pacc{h}"]
            if jt == 0:
                nc.vector.tensor_copy(pacc[:], P[:])
            else:
                nc.vector.tensor_tensor(
                    pacc[:, off:512], pacc[:, off:512], P[:, off:512], add
                )

        def att_iter(h, jt):
            if jt == 0:
                start_h(h)
            off = max(0, 128 * jt - 512 * s)
            pss = ppa.tile([128, 512], f32, tag="aps", name="pss")
            nc.tensor.matmul(
                pss[:, off:512],
                qk_sb[:, GQ, jt * 128 : (jt + 1) * 128],
                qk_sb[:, h, s * 512 + off : (s + 1) * 512],
                start=True,
                stop=True,
            )
            P = ptp.tile([128, 512], bf, tag="P", name="P")
            nc.scalar.activation(
                P[:, off:512], pss[:, off:512], Exp, scale=1.0 / 128.0
            )
            if jt >= 4 * s:
                nc.gpsimd.affine_select(
                    out=P[:, off : off + 128],
                    in_=P[:, off : off + 128],
                    pattern=[[1, 128]],
                    compare_op=mybir.AluOpType.is_ge,
                    fill=0.0,
                    base=0,
                    channel_multiplier=-1,
                )
            while len(st["pend"]) >= PIPE:
                drain_one()
            st["pend"].append((P, h, jt, off))

        def normalize(h):
            while st["pend"]:
                drain_one()
            psd = ppo.tile([128, 512], f32, tag="ops", name="psd")
            nc.tensor.matmul(
                psd[:], ones_sb[:], st[f"pacc{h}"][:], start=True, stop=True
            )
            rinv = smp.tile([128, 512], f32, tag="rinv", name="rinv")
            nc.vector.reciprocal_approx_fast(rinv[:], psd[:])
            nc.vector.tensor_tensor(y_sb[:, h, isl], st["psy"][:], rinv[:], mul)

        for h in range(GQ):
            for jt in range(njt):
                thunks.append(lambda h=h, jt=jt: att_iter(h, jt))
            thunks.append(lambda h=h: normalize(h))
        return thunks

    def op_stream(s):
        st = {}
        thunks = []

        def group(tt, es):
            if es == 0:
                st["osb"] = osp.tile([128, C], f32, tag="osb", name="osb")
            pso = ppo.tile([128, 512], f32, tag="ops", name="pso")
            for h in range(GQ):
                nc.tensor.matmul(
                    pso[:],
                    y_sb[:, h, tt * 128 : (tt + 1) * 128],
                    wout_sb[:, h, es * 512 : (es + 1) * 512],
                    start=(h == 0),
                    stop=(h == GQ - 1),
                )
            o_sb = st["osb"]
            osl = slice(es * 512, (es + 1) * 512)
            if es % 2 == 0:
                nc.vector.tensor_copy(o_sb[:, osl], pso[:])
            else:
                nc.scalar.copy(o_sb[:, osl], pso[:])
            nc.sync.dma_start(out_d[tt * 128 : (tt + 1) * 128, osl], o_sb[:, osl])

        for tt in range(4 * s, 4 * s + 4):
            for es in range(4):
                thunks.append(lambda tt=tt, es=es: group(tt, es))
        return thunks

    def run_interleaved(*streams):
        # proportional (Bresenham) merge; streams are lists of thunks
        streams = [list(s) for s in streams if s]
        total = max(len(s) for s in streams)
        err = [0.0] * len(streams)
        idx = [0] * len(streams)
        for _ in range(total):
            for k, s in enumerate(streams):
                err[k] += len(s) / total
                while err[k] >= 1.0 and idx[k] < len(s):
                    s[idx[k]]()
                    idx[k] += 1
                    err[k] -= 1.0
        for k, s in enumerate(streams):
            while idx[k] < len(s):
                s[idx[k]]()
                idx[k] += 1

    # slice-level software pipeline: attention(s) overlaps QKV(s+1);
    # out-projections ride in the late (ACT-bound) attention slices
    run_interleaved(qkv_stream(0))
    run_interleaved(att_stream(0), qkv_stream(1))
    run_interleaved(att_stream(1), qkv_stream(2))
    run_interleaved(att_stream(2), qkv_stream(3), op_stream(0))
    run_interleaved(att_stream(3), op_stream(1), op_stream(2))
    run_interleaved(op_stream(3))


def _host_prep(x, rope_cache, W_qkv, b_qkv, W_out):
    """Build the 8 per-core input dicts."""
    import concourse.mybir as mybir

    F8 = mybir.dt.np(mybir.dt.float8e4)
    q_dim = NH * D  # 2048
    kv_dim = NKV * D  # 512

    # rope tables in [d, t] layout
    sin = rope_cache[:, 0::2].astype(np.float32)  # [T, 64]
    cos = rope_cache[:, 1::2].astype(np.float32)
    cos2T = np.empty((128, T), np.float32)
    sinsT = np.empty((128, T), np.float32)
    cos2T[0::2] = cos.T
    cos2T[1::2] = cos.T
    sinsT[0::2] = -sin.T
    sinsT[1::2] = sin.T
    cos2T = cos2T.astype(BF16)
    sinsT = sinsT.astype(BF16)

    swap = np.zeros((128, 128), BF16)
    idx = np.arange(128)
    swap[idx, idx ^ 1] = 1
    iden = np.eye(128, dtype=BF16)

    def tile_cols(w, ncols):
        # [C, ncols*128] -> [128, CC, ncols*128] with contraction c = cc*128+p
        return np.ascontiguousarray(
            w.reshape(CC, 128, ncols * 128).transpose(1, 0, 2)
        )

    in_maps = []
    for b in range(B):
        xT = np.ascontiguousarray(x[b].T)  # [C, T] f32
        xT = xT.reshape(CC, 128, T).transpose(1, 0, 2)  # [128, CC, T]
        # -> [128, TS, CC, 512]: per-slice DMA reads contiguous lines
        xT = xT.reshape(128, CC, TS, 512).transpose(0, 2, 1, 3)
        xt8 = np.ascontiguousarray(np.clip(xT, -240, 240).astype(F8))
        xtb = np.ascontiguousarray(xT.astype(BF16))
        for g in range(NKV):
            qk_cols = np.concatenate(
                [
                    np.arange(4 * g * D, (4 * g + 4) * D),  # 4 q heads
                    np.arange(q_dim + g * D, q_dim + (g + 1) * D),  # k head
                ]
            )
            v_cols = np.arange(q_dim + kv_dim + g * D, q_dim + kv_dim + (g + 1) * D)
            wqk = np.clip(W_qkv[:, qk_cols] * WSCALE, -240, 240).astype(F8)
            wv = W_qkv[:, v_cols].astype(BF16)
            bq = np.ascontiguousarray(
                b_qkv[np.concatenate([qk_cols, v_cols])]
                .astype(np.float32).reshape(NQK + 1, 128).T
            )  # [128, NQK+1]
            wo = W_out[4 * g * D : (4 * g + 4) * D, :].astype(BF16)  # [512, C]
            wo = np.ascontiguousarray(
                wo.reshape(GQ, 128, C).transpose(1, 0, 2)
            )  # [128, GQ, C]
            in_maps.append(
                {
                    "xt8": xt8,
                    "xtb": xtb,
                    "wqk8": tile_cols(wqk, NQK),
                    "wv": tile_cols(wv, 1),
                    "bqkv": bq,
                    "cosT": cos2T,
                    "sinT": sinsT,
                    "swp": swap,
                    "idn": iden,
                    "wout": wo,
                }
            )
    return in_maps


def kernel(x, rope_cache, W_qkv, b_qkv, W_out, b_out, _trace=False):
    from concourse.bass_utils import run_bass_kernel_spmd

    if "nc" not in _CACHED:
        _CACHED["nc"] = _build_bass()
    nc = _CACHED["nc"]

    in_maps = _host_prep(
        np.asarray(x), np.asarray(rope_cache), np.asarray(W_qkv),
        np.asarray(b_qkv), np.asarray(W_out),
    )
    res = run_bass_kernel_spmd(nc, in_maps, core_ids=list(range(N_CORES)), trace=_trace)
    _CACHED["last_result"] = res

    out = np.zeros((B, T, C), np.float32)
    for b in range(B):
        acc = res.results[b * NKV]["out"].astype(np.float32)
        for g in range(1, NKV):
            acc = acc + res.results[b * NKV + g]["out"]
        out[b] = acc + np.asarray(b_out)[None, :]
    return out
